# revision 4
# baseline (speedup 1.0000x reference)
"""Trainium2 Bass kernel for nn_DPXEmbedder (ragged ViT-style embedder).

Math: the reference projects fV [N,T] @ W_proj [T,D] + b_proj, ragged-gathers
rows into per-image padded sequences (cls token first), then LayerNorms every
row.  Because the per-batch unique-key machinery reduces to contiguous row
ranges of fV, the device work is exactly `LN(fV @ W + b)` over the N rows.
Pad rows LayerNorm to ln_beta identically; the cls row is a single 768-elem
LN done on host.

Sharding: data-parallel over rows.  N=30500 rows padded to 30720 = 8 cores x
3840 rows.  Each core runs an identical SPMD program: 30 M-tiles x (8 K-tiles
x 2 matmuls) with fp16 operands accumulating fp32 in PSUM, then bn_stats/
bn_aggr + scalar-engine apply for the LayerNorm, and writes [3840, 768] fp32.
"""

import numpy as np
from contextlib import ExitStack

import concourse.bass as bass
import concourse.tile as tile
from concourse import mybir
from concourse.bass_utils import run_bass_kernel_spmd

# ---- hardcoded problem shapes ----
T = 1024          # proj in-dim (K)
D = 768           # embed dim (N of GEMM)
N_CORES = 8
M_TILES = 30      # 3840 rows / 128
ROWS_PER_CORE = M_TILES * 128   # 3840
K_TILES = T // 128              # 8
EPS = 1e-5
NSPLIT = (512, 256)             # D split across two PSUM banks

_F16 = mybir.dt.float16
_F32 = mybir.dt.float32


def _legalize_waits(nc):
    """This walrus build encodes at most ONE sem wait per instruction.
    bass_rust emits multi-wait sync_info freely, so split the extras onto
    preceding single-wait NoOps on the same engine (sequential waiting on
    monotonic sems is equivalent to the conjunction)."""
    for f in nc.m.functions:
        for blk in f.blocks:
            newl = []
            for inst in blk.instructions:
                si = inst.sync_info
                if si is not None and si.on_wait and len(si.on_wait) > 1:
                    waits = list(si.on_wait)
                    for w in waits[:-1]:
                        newl.append(mybir.InstNoOp(
                            name=nc.get_next_instruction_name(),
                            engine=inst.engine,
                            sync_info=mybir.SyncInfo(on_wait=[w], on_update=[]),
                        ))
                    si.on_wait = [waits[-1]]
                newl.append(inst)
            blk.instructions = newl


def _build_nc(with_bias: bool, with_gamma: bool, with_beta: bool):
    nc = bass.Bass()
    fvt = nc.dram_tensor("fvt", [M_TILES, 128, K_TILES, 128], _F16,
                         kind="ExternalInput")
    w = nc.dram_tensor("w", [T, D], _F16, kind="ExternalInput")
    out = nc.dram_tensor("out", [ROWS_PER_CORE, D], _F32, kind="ExternalOutput")
    if with_bias:
        bvec = nc.dram_tensor("bvec", [1, D], _F16, kind="ExternalInput")
    if with_gamma:
        gvec = nc.dram_tensor("gvec", [D], _F32, kind="ExternalInput")
    if with_beta:
        bevec = nc.dram_tensor("bevec", [D], _F32, kind="ExternalInput")

    with tile.TileContext(nc) as tc, ExitStack() as ctx:
        singles = ctx.enter_context(tc.tile_pool(name="singles", bufs=1))
        xp = ctx.enter_context(tc.tile_pool(name="xp", bufs=4))
        pp = ctx.enter_context(tc.tile_pool(name="pp", bufs=3, space="PSUM"))
        op = ctx.enter_context(tc.tile_pool(name="op", bufs=3))
        sp = ctx.enter_context(tc.tile_pool(name="sp", bufs=4))

        w_sb = singles.tile([128, K_TILES, D], _F16)
        nc.sync.dma_start(out=w_sb, in_=w.rearrange("(ko p) n -> p ko n", p=128))
        eps_t = singles.tile([128, 1], _F32)
        nc.vector.memset(eps_t, EPS)
        if with_bias:
            ones_t = singles.tile([1, 128], _F16)
            nc.vector.memset(ones_t, 1.0)
            b_sb = singles.tile([1, D], _F16)
            nc.sync.dma_start(out=b_sb, in_=bvec[:, :])
        if with_gamma:
            g_sb = singles.tile([128, D], _F32)
            nc.sync.dma_start(out=g_sb, in_=bass.AP(
                tensor=gvec.tensor, offset=gvec.offset,
                ap=[[0, 128]] + list(gvec.ap)))
        if with_beta:
            be_sb = singles.tile([128, D], _F32)
            nc.sync.dma_start(out=be_sb, in_=bass.AP(
                tensor=bevec.tensor, offset=bevec.offset,
                ap=[[0, 128]] + list(bevec.ap)))

        for mt in range(M_TILES):
            xt = xp.tile([128, K_TILES, 128], _F16, tag="xt")
            nc.sync.dma_start(out=xt, in_=fvt[mt])

            pa = pp.tile([128, NSPLIT[0]], _F32, tag="pa")
            pb = pp.tile([128, NSPLIT[1]], _F32, tag="pb")
            for ko in range(K_TILES):
                last = (ko == K_TILES - 1) and not with_bias
                nc.tensor.matmul(pa, lhsT=xt[:, ko, :],
                                 rhs=w_sb[:, ko, 0:NSPLIT[0]],
                                 start=(ko == 0), stop=last)
                nc.tensor.matmul(pb, lhsT=xt[:, ko, :],
                                 rhs=w_sb[:, ko, NSPLIT[0]:D],
                                 start=(ko == 0), stop=last)
            if with_bias:
                nc.tensor.matmul(pa, lhsT=ones_t, rhs=b_sb[:, 0:NSPLIT[0]],
                                 start=False, stop=True)
                nc.tensor.matmul(pb, lhsT=ones_t, rhs=b_sb[:, NSPLIT[0]:D],
                                 start=False, stop=True)

            # LayerNorm stats: one bn_stats per PSUM bank, aggregate, rstd.
            stats = sp.tile([128, 2, 6], _F32, tag="stats")
            nc.vector.bn_stats(out=stats[:, 0, :], in_=pa)
            nc.vector.bn_stats(out=stats[:, 1, :], in_=pb)
            mv = sp.tile([128, 2], _F32, tag="mv")
            nc.vector.bn_aggr(out=mv, in_=stats)
            rstd = sp.tile([128, 1], _F32, tag="rstd")
            nc.scalar.activation(out=rstd, in_=mv[:, 1:2],
                                 func=mybir.ActivationFunctionType.Sqrt,
                                 bias=eps_t, scale=1.0)
            nc.vector.reciprocal(out=rstd, in_=rstd)
            nmr = sp.tile([128, 1], _F32, tag="nmr")
            nc.vector.tensor_scalar(out=nmr, in0=mv[:, 0:1], scalar1=rstd,
                                    scalar2=-1.0, op0=mybir.AluOpType.mult,
                                    op1=mybir.AluOpType.mult)

            # apply (x - mu) * rstd on the scalar engine: Identity(scale*x+bias)
            osb = op.tile([128, D], _F32, tag="osb")
            nc.scalar.activation(out=osb[:, 0:NSPLIT[0]], in_=pa,
                                 func=mybir.ActivationFunctionType.Identity,
                                 bias=nmr, scale=rstd)
            nc.scalar.activation(out=osb[:, NSPLIT[0]:D], in_=pb,
                                 func=mybir.ActivationFunctionType.Identity,
                                 bias=nmr, scale=rstd)
            if with_gamma:
                nc.vector.tensor_mul(out=osb, in0=osb, in1=g_sb)
            if with_beta:
                nc.vector.tensor_add(out=osb, in0=osb, in1=be_sb)

            # out-DMA issued on the Activation engine: follows its own apply
            # ops in program order, avoiding cross-engine waits.
            nc.scalar.dma_start(out=out[mt * 128:(mt + 1) * 128, :], in_=osb)

    _legalize_waits(nc)
    return nc


_NC_CACHE = {}


def _get_nc(flags):
    if flags not in _NC_CACHE:
        _NC_CACHE[flags] = _build_nc(*flags)
    return _NC_CACHE[flags]


def _make_in_maps(inputs, flags):
    fV = np.asarray(inputs["fV"], dtype=np.float32)
    W = np.asarray(inputs["W_proj"], dtype=np.float32)
    b = np.asarray(inputs["b_proj"], dtype=np.float32)
    g = np.asarray(inputs["ln_gamma"], dtype=np.float32)
    be = np.asarray(inputs["ln_beta"], dtype=np.float32)
    N = fV.shape[0]

    total = N_CORES * ROWS_PER_CORE
    fv16 = np.zeros((total, T), dtype=np.float16)
    fv16[:N] = fV.astype(np.float16)
    w16 = np.ascontiguousarray(W.astype(np.float16))

    in_maps = []
    for c in range(N_CORES):
        shard = fv16[c * ROWS_PER_CORE:(c + 1) * ROWS_PER_CORE]
        # [3840, 1024] -> [mt, ki, ko, mi] with fV[mt*128+mi, ko*128+ki]
        tiled = np.ascontiguousarray(
            shard.reshape(M_TILES, 128, K_TILES, 128).transpose(0, 3, 2, 1))
        m = {"fvt": tiled, "w": w16}
        if flags[0]:
            m["bvec"] = b.astype(np.float16).reshape(1, D)
        if flags[1]:
            m["gvec"] = g
        if flags[2]:
            m["bevec"] = be
        in_maps.append(m)
    return in_maps


def _host_layernorm(x, g, b, eps=EPS):
    mu = x.mean(-1, keepdims=True)
    var = ((x - mu) ** 2).mean(-1, keepdims=True)
    return (x - mu) / np.sqrt(var + eps) * g + b


def kernel(**inputs):
    fV = np.asarray(inputs["fV"], dtype=np.float32)
    seg = np.asarray(inputs["seg"])
    byx0 = np.asarray(inputs["byx0"])
    W = np.asarray(inputs["W_proj"], dtype=np.float32)
    b = np.asarray(inputs["b_proj"], dtype=np.float32)
    cls_token = np.asarray(inputs["cls_token"], dtype=np.float32)
    g = np.asarray(inputs["ln_gamma"], dtype=np.float32)
    be = np.asarray(inputs["ln_beta"], dtype=np.float32)

    Bn = seg.shape[0]
    N, Tdim = fV.shape
    nC, Dm = cls_token.shape
    assert Tdim == T and Dm == D

    # per-batch token counts / offsets, mirroring the reference's unique-key math
    keys = seg.reshape(-1).astype(np.int64) * Bn + byx0.astype(np.int64)
    uniq = np.unique(keys)
    if uniq.size > N:
        uniq = uniq[:N]
    elif uniq.size < N:
        uniq = np.pad(uniq, (0, N - uniq.size), constant_values=int(uniq.min()))
    b_idx = (uniq % Bn).astype(np.int64)
    bc = np.bincount(b_idx, minlength=Bn)
    st = np.cumsum(bc) - bc
    maxdim = int(bc.max()) + 1

    # ---- device GEMM + LN over all N rows (padded to 8*3840) ----
    flags = (bool(np.any(b != 0.0)), bool(np.any(g != 1.0)), bool(np.any(be != 0.0)))
    nc = _get_nc(flags)
    in_maps = _make_in_maps(
        dict(fV=fV, W_proj=W, b_proj=b, ln_gamma=g, ln_beta=be), flags)

    res = run_bass_kernel_spmd(nc, in_maps, core_ids=list(range(N_CORES)))
    rows = np.concatenate([res.results[c]["out"] for c in range(N_CORES)], axis=0)[:N]

    # ---- host assembly ----
    out = np.empty((Bn, maxdim, D), dtype=np.float32)
    cls_ln = _host_layernorm(cls_token, g, be).astype(np.float32)  # [nC, D]
    for bi in range(Bn):
        c = int(bc[bi])
        out[bi, 0:nC] = cls_ln
        out[bi, nC:nC + c] = rows[int(st[bi]):int(st[bi]) + c]
        out[bi, nC + c:] = be  # LN(0-row) == beta exactly
    amask = np.arange(maxdim)[None, :] < (bc + nC)[:, None]
    return out, seg, amask
